# revision 1
# baseline (speedup 1.0000x reference)
"""Trainium2 Bass kernel for nn_DenoiserBlock (B=2, L=2048, D=1024, H=16, F=4096).

Sharding: 8 cores = 2 (batch) x 4 (query-slice of 512). Each core computes
K/V for the full sequence of its batch element (no collectives), attention +
MLP for its 512-query slice. Host does AdaLN precompute, weight re-layout and
fp8 quantization, and final concatenation of the 8 [512, 1024] output slices.

Device dataflow (per core):
  A: LN1 over x[b] (bf16, pipelined groups of 4 row tiles) -> PE-transpose ->
     hT (fp8 DoubleRow pair layout); same for the 512 query rows -> hqT
  B: fp8 DoubleRow projections -> qT (fp8, zero-padded pair layout),
     kT (fp8), v (fp8, pair layout, appended ones column per head)
  C: per head: scores = DR(kT,qT) + DR(sc_h*I, -torus) accumulated in psum;
     wide exp(s - 5) (ACT) -> pm fp8; attn@v via fp8 DR over key-tile pairs
     (denominator in column 64); normalize*16, PE-transpose -> outT (fp8)
  D: x2 = DR(outT, Wout)/512 + xres; LN2 -> h2T (fp8) + dh2T (fp8 residual)
  E: aT = 8*gelu(DR(w1, h2T) + DR(dw1, h2T) + DR(w1, dh2T) + b1) (fp8);
     y = (DR(aT, w2) + DR(aT, dw2))/(8*SW) + x2   (b2 rides a w2 slot)

Algebraic folds (host): k-bias dropped (softmax shift-invariance per query),
LN gains folded into weights, mask+torus bias folded into an fp8 log-bias
tensor injected into the scores psum by an identity matmul, exp shifted by -5
(softmax-invariant) to keep fp8 probabilities in range, W1/W2 carry fp8
residual-compensation slots, b2 rides an extra W2 contraction slot.
"""

import sys

sys.path.insert(0, "/opt/trn_rl_repo")

import numpy as np
import ml_dtypes

import concourse.bacc as bacc
import concourse.mybir as mybir
from concourse import tile, masks
from concourse.bass_utils import run_bass_kernel_spmd

F32 = mybir.dt.float32
BF16 = mybir.dt.bfloat16
FP8 = mybir.dt.float8e4
AX = mybir.AxisListType
OP = mybir.AluOpType
ACT = mybir.ActivationFunctionType
PM = mybir.MatmulPerfMode

B, L, D, H, F = 2, 2048, 1024, 16, 4096
HD = D // H          # 64
QS = 512             # queries per core
NC_PER_B = 4
NLT = L // 128       # 16
NDT = D // 128       # 8
NQT = QS // 128      # 4
NFT = F // 128       # 32
EPS = 1e-5
SW = 32.0            # fp8 weight upscale
SA = 16.0            # attn-out upscale
SG = 8.0             # gelu-out upscale
ESH = -5.0           # softmax exp shift

_CACHED = {}


def _build(allmask=True):
    nc = bacc.Bacc("TRN2", target_bir_lowering=False, debug=False, num_devices=8)

    d_xt = nc.dram_tensor("xt", [128, NLT, D], BF16, kind="ExternalInput")
    d_xres = nc.dram_tensor("xres", [128, NQT, D], F32, kind="ExternalInput")
    d_tor = nc.dram_tensor("tor", [128, NLT + 1, QS], FP8, kind="ExternalInput")
    d_iddr = nc.dram_tensor("iddr", [128, H, 2, 128], FP8, kind="ExternalInput")
    d_wq = nc.dram_tensor("wq", [128, 4, 2, D], FP8, kind="ExternalInput")
    d_wk = nc.dram_tensor("wk", [128, 4, 2, D], FP8, kind="ExternalInput")
    d_wv = nc.dram_tensor("wv", [128, 4, 2, D], FP8, kind="ExternalInput")
    d_wo = nc.dram_tensor("wo", [128, 4, 2, D], FP8, kind="ExternalInput")
    d_w1 = nc.dram_tensor("w1", [128, 8, 8, 2, 512], FP8, kind="ExternalInput")
    d_w2 = nc.dram_tensor("w2", [128, 34, 2, D], FP8, kind="ExternalInput")
    d_biasq = nc.dram_tensor("biasq", [128, NDT], F32, kind="ExternalInput")
    d_b1sb = nc.dram_tensor("b1sb", [128, NFT], F32, kind="ExternalInput")
    d_km = nc.dram_tensor("km", [128, NLT], F32, kind="ExternalInput")
    d_bvrep = nc.dram_tensor("bvrep", [128, D], F32, kind="ExternalInput")
    d_y = nc.dram_tensor("y", [128, NQT, D], F32, kind="ExternalOutput")

    with tile.TileContext(nc) as tc:
        with (
            tc.tile_pool(name="const", bufs=1) as cpool,
            tc.tile_pool(name="mid", bufs=1) as mpool,
            tc.tile_pool(name="psum", bufs=1, space="PSUM") as pspool,
        ):
            identb = cpool.tile([128, 128], BF16, tag="identb")
            epsc = cpool.tile([128, 1], F32, tag="epsc")
            eshc = cpool.tile([128, 1], F32, tag="eshc")
            biasq = cpool.tile([128, NDT], F32, tag="biasq")
            b1sb = cpool.tile([128, NFT], F32, tag="b1sb")
            km = cpool.tile([128, NLT], F32, tag="km")
            masks.make_identity(nc, identb[:])
            nc.vector.memset(epsc[:], EPS)
            nc.vector.memset(eshc[:], ESH)
            nc.sync.dma_start(biasq[:], d_biasq[:])
            nc.sync.dma_start(b1sb[:], d_b1sb[:])
            nc.sync.dma_start(km[:], d_km[:])

            x2 = mpool.tile([128, NQT, D], F32, tag="x2", name="x2")
            outT = mpool.tile([128, NDT, QS], FP8, tag="outT", name="outT")
            wo = mpool.tile([128, 4, 2, D], FP8, tag="wo", name="wo")
            h2T = mpool.tile([128, NDT, QS], FP8, tag="h2T", name="h2T")
            dh2T = mpool.tile([128, NDT, QS], FP8, tag="dh2T", name="dh2T")

            def ln_stats(pool, xt_ap, s1c, s2c, eng=None):
                sq = pool.tile([128, D], BF16, tag="sq", name="sq", bufs=3)
                (eng or nc.vector).tensor_reduce(s1c, xt_ap, axis=AX.X,
                                                 op=OP.add)
                nc.scalar.activation(sq[:], xt_ap, ACT.Square, accum_out=s2c)

            def ln_finalize(pool, s1a, s2a, n, rstd_a, nmr_a, tag):
                mu = pool.tile([128, n], F32, tag=tag, name="mu", bufs=10)
                ms = pool.tile([128, n], F32, tag=tag, name="ms", bufs=10)
                var = pool.tile([128, n], F32, tag=tag, name="var", bufs=10)
                std = pool.tile([128, n], F32, tag=tag, name="std", bufs=10)
                nc.vector.tensor_scalar(mu[:], s1a, 1.0 / D, None, op0=OP.mult)
                nc.vector.tensor_tensor(ms[:], mu[:], mu[:], op=OP.mult)
                nc.vector.scalar_tensor_tensor(
                    var[:], s2a, 1.0 / D, ms[:], op0=OP.mult, op1=OP.subtract)
                nc.scalar.activation(std[:], var[:], ACT.Sqrt, bias=epsc[:])
                nc.vector.reciprocal(rstd_a, std[:])
                nc.vector.scalar_tensor_tensor(
                    nmr_a, mu[:], -1.0, rstd_a, op0=OP.mult, op1=OP.mult)

            def ln_norm_transpose(pool, xt_ap, rstd_c, nmr_c, dstT, col0,
                                  ddstT=None, psp=None, heng=None):
                hb = pool.tile([128, D], BF16, tag="hb", name="hb", bufs=3)
                (heng or nc.vector).tensor_scalar(hb[:], xt_ap, rstd_c, nmr_c,
                                                  op0=OP.mult, op1=OP.add)
                for half in range(2):
                    pst = psp.tile([128, 512], BF16, tag="trp", name="pst",
                                   bufs=2)
                    for j in range(4):
                        dt_ = half * 4 + j
                        nc.tensor.transpose(
                            pst[:, j * 128:(j + 1) * 128],
                            hb[:, dt_ * 128:(dt_ + 1) * 128], identb[:])
                    p3 = pst[:].rearrange("p (a b) -> p a b", b=128)
                    dst = dstT[:, half * 4:half * 4 + 4, col0:col0 + 128]
                    if half == 0:
                        nc.vector.tensor_copy(dst, p3)
                    else:
                        nc.scalar.activation(dst, p3, ACT.Copy)
                    if ddstT is not None:
                        nc.vector.tensor_tensor(
                            ddstT[:, half * 4:half * 4 + 4, col0:col0 + 128],
                            p3, dst, op=OP.subtract)

            with (
                tc.tile_pool(name="attn", bufs=1) as atpool,
                tc.tile_pool(name="psat", bufs=1, space="PSUM") as psat,
            ):
                kT = atpool.tile([128, NDT + 1, L], FP8, tag="kT", name="kT")
                qT = atpool.tile([128, NDT, 2, QS], FP8, tag="qT", name="qT")
                vv = atpool.tile([128, NLT // 2, 2, H, HD + 1], FP8,
                                 tag="vv", name="vv")
                tor = atpool.tile([128, NLT + 1, QS], FP8, tag="tor", name="tor")
                iddr = atpool.tile([128, H, 2, 128], FP8, tag="iddr",
                                   name="iddr")
                nc.gpsimd.memset(kT[:, NDT, :], 0.0)
                nc.gpsimd.memset(qT[:, :, 1, :], 0.0)
                nc.gpsimd.memset(vv[:, :, :, :, HD], 1.0)

                # ---- Phase A ----
                with tc.tile_pool(name="hTp", bufs=1) as hpool:
                    hT = hpool.tile([128, NDT, L], FP8, tag="hT", name="hT")
                    hqT = hpool.tile([128, NDT, QS], FP8, tag="hqT", name="hqT")
                    with tc.tile_pool(name="phA", bufs=1) as apool:
                        xt = apool.tile([128, NLT, D], BF16, tag="xt", name="xt")
                        xres = apool.tile([128, NQT, D], F32, tag="xres",
                                          name="xres")
                        for c in range(4):
                            nc.sync.dma_start(xt[:, c * 4:(c + 1) * 4, :],
                                              d_xt[:, c * 4:(c + 1) * 4, :])
                        nc.sync.dma_start(xres[:], d_xres[:])
                        s1a = apool.tile([128, NLT], F32, tag="s1a", name="s1a")
                        s2a = apool.tile([128, NLT], F32, tag="s2a", name="s2a")
                        rstd = apool.tile([128, NLT], F32, tag="rstd",
                                          name="rstd")
                        nmr = apool.tile([128, NLT], F32, tag="nmr", name="nmr")
                        for g in range(4):
                            s_ = slice(4 * g, 4 * g + 4)
                            for lt in range(4 * g, 4 * g + 4):
                                ln_stats(apool, xt[:, lt, :],
                                         s1a[:, lt:lt + 1], s2a[:, lt:lt + 1])
                            ln_finalize(apool, s1a[:, s_], s2a[:, s_], 4,
                                        rstd[:, s_], nmr[:, s_], "lnfA")
                            for lt in range(4 * g, 4 * g + 4):
                                ln_norm_transpose(apool, xt[:, lt, :],
                                                  rstd[:, lt:lt + 1],
                                                  nmr[:, lt:lt + 1], hT,
                                                  lt * 128, psp=psat,
                                                  heng=(None if lt % 2 == 0
                                                        else nc.gpsimd))
                        s1q = apool.tile([128, NQT], F32, tag="s1q", name="s1q")
                        s2q = apool.tile([128, NQT], F32, tag="s2q", name="s2q")
                        rstdq = apool.tile([128, NQT], F32, tag="rstdq",
                                           name="rstdq")
                        nmrq = apool.tile([128, NQT], F32, tag="nmrq",
                                          name="nmrq")
                        for qt in range(NQT):
                            ln_stats(apool, xres[:, qt, :], s1q[:, qt:qt + 1],
                                     s2q[:, qt:qt + 1])
                        ln_finalize(apool, s1q[:], s2q[:], NQT, rstdq[:],
                                    nmrq[:], "lnfA")
                        for qt in range(NQT):
                            ln_norm_transpose(apool, xres[:, qt, :],
                                              rstdq[:, qt:qt + 1],
                                              nmrq[:, qt:qt + 1], hqT,
                                              qt * 128, psp=psat)

                    # ---- Phase B ----
                    with (
                        tc.tile_pool(name="wtsB", bufs=1) as wbpool,
                        tc.tile_pool(name="psB", bufs=1, space="PSUM") as psB,
                    ):
                        wq = wbpool.tile([128, 4, 2, D], FP8, tag="wq",
                                         name="wq")
                        wk = wbpool.tile([128, 4, 2, D], FP8, tag="wk",
                                         name="wk")
                        wv = wbpool.tile([128, 4, 2, D], FP8, tag="wv",
                                         name="wv")
                        bvrep = wbpool.tile([128, D], F32, tag="bvrep",
                                            name="bvrep")
                        nc.sync.dma_start(wq[:], d_wq[:])
                        nc.sync.dma_start(wk[:], d_wk[:])
                        nc.sync.dma_start(wv[:], d_wv[:])
                        nc.sync.dma_start(bvrep[:], d_bvrep[:])
                        nc.sync.dma_start(tor[:], d_tor[:])
                        nc.sync.dma_start(iddr[:], d_iddr[:])
                        nc.sync.dma_start(wo[:], d_wo[:])

                        for i in range(NDT):
                            pq = psB.tile([128, 512], F32, tag="mmb",
                                          name="pq", bufs=4)
                            for j in range(4):
                                nc.tensor.matmul(
                                    pq[:, 0:512],
                                    wq[:, j, :, i * 128:(i + 1) * 128],
                                    hqT[:, 2 * j:2 * j + 2, :],
                                    start=(j == 0), stop=(j == 3),
                                    perf_mode=PM.DoubleRow)
                            nc.scalar.activation(
                                qT[:, i, 0, :], pq[:, 0:512], ACT.Identity,
                                bias=biasq[:, i:i + 1], scale=1.0 / SW)
                            for cc in range(4):
                                pk = psB.tile([128, 512], F32, tag="mmb",
                                              name="pk", bufs=4)
                                c0 = cc * 512
                                for j in range(4):
                                    nc.tensor.matmul(
                                        pk[:],
                                        wk[:, j, :, i * 128:(i + 1) * 128],
                                        hT[:, 2 * j:2 * j + 2, c0:c0 + 512],
                                        start=(j == 0), stop=(j == 3),
                                        perf_mode=PM.DoubleRow)
                                nc.scalar.activation(
                                    kT[:, i, cc * 512:(cc + 1) * 512], pk[:],
                                    ACT.Copy, scale=1.0 / SW)
                            for lt in range(2 * i, 2 * i + 2):
                                for half in range(2):
                                    pv = psB.tile([128, 512], F32, tag="mmb",
                                                  name="pv", bufs=4)
                                    for j in range(4):
                                        nc.tensor.matmul(
                                            pv[:],
                                            hT[:, 2 * j:2 * j + 2,
                                               lt * 128:(lt + 1) * 128],
                                            wv[:, j, :,
                                               half * 512:(half + 1) * 512],
                                            start=(j == 0), stop=(j == 3),
                                            perf_mode=PM.DoubleRow)
                                    h0 = half * 8
                                    nc.vector.scalar_tensor_tensor(
                                        vv[:, lt // 2, lt % 2, h0:h0 + 8, 0:HD],
                                        pv[:].rearrange("p (h c) -> p h c",
                                                        c=HD),
                                        1.0 / SW,
                                        bvrep[:, half * 512:(half + 1) * 512]
                                        .rearrange("p (h c) -> p h c", c=HD),
                                        op0=OP.mult, op1=OP.add)
                # ---- Phase C ----
                with (
                    tc.tile_pool(name="phC", bufs=1) as cpool2,
                    tc.tile_pool(name="psC", bufs=1, space="PSUM") as psC,
                ):
                    for i in range(NDT):
                        ptp = psat.tile([128, 512], BF16, tag="trp",
                                        name="ptp", bufs=2)
                        pt3 = ptp[:].rearrange("p (a b) -> p a b", b=128)
                        for par in range(2):
                            h = 2 * i + par
                            p0 = 64 * par
                            acc = psat.tile([128, NQT, HD + 1], F32,
                                            tag="acc", name="acc", bufs=2)
                            pms = []
                            for jj in range(NLT // 2):
                                ps = psC.tile([128, 1024], F32, tag="mmc",
                                              name="ps", bufs=2)
                                for t in range(2):
                                    ktt = 2 * jj + t
                                    sl = slice(t * 512, (t + 1) * 512)
                                    nc.tensor.matmul(
                                        ps[:, sl],
                                        kT[p0:p0 + 64, i:i + 2,
                                           ktt * 128:(ktt + 1) * 128],
                                        qT[p0:p0 + 64, i, :, :],
                                        start=True, stop=False,
                                        perf_mode=PM.DoubleRow,
                                        skip_group_check=True)
                                    nc.tensor.matmul(
                                        ps[:, sl], iddr[:, h, :, :],
                                        tor[:, ktt:ktt + 2, :],
                                        start=False, stop=True,
                                        perf_mode=PM.DoubleRow,
                                        skip_group_check=True)
                                pm2 = cpool2.tile([128, 2, 512], FP8,
                                                  tag="pm2", name="pm2",
                                                  bufs=10)
                                if allmask:
                                    nc.scalar.activation(
                                        pm2[:].rearrange("p a b -> p (a b)"),
                                        ps[:], ACT.Exp, bias=eshc[:])
                                else:
                                    for t in range(2):
                                        nc.scalar.activation(
                                            pm2[:, t, :],
                                            ps[:, t * 512:(t + 1) * 512],
                                            ACT.Exp,
                                            bias=km[:, 2 * jj + t:
                                                    2 * jj + t + 1])
                                pms.append(pm2)
                            for jj in range(NLT // 2):
                                for qt in range(NQT):
                                    nc.tensor.matmul(
                                        acc[:, qt, :],
                                        pms[jj][:, :, qt * 128:(qt + 1) * 128],
                                        vv[:, jj, :, h, :],
                                        start=(jj == 0),
                                        stop=(jj == NLT // 2 - 1),
                                        perf_mode=PM.DoubleRow)
                            rs = cpool2.tile([128, NQT], F32, tag="rs",
                                             name="rs", bufs=4)
                            rc = cpool2.tile([128, NQT], F32, tag="rs",
                                             name="rc", bufs=4)
                            nc.vector.tensor_scalar(
                                rs[:], acc[:, :, HD], 1e-30, None, op0=OP.add)
                            nc.vector.reciprocal(rc[:], rs[:])
                            for qt in range(NQT):
                                asb = cpool2.tile([128, HD], BF16, tag="asb",
                                                  name="asb", bufs=4)
                                nc.vector.tensor_scalar(
                                    asb[:], acc[:, qt, 0:HD], rc[:, qt:qt + 1],
                                    SA, op0=OP.mult, op1=OP.mult)
                                nc.tensor.transpose(
                                    pt3[p0:p0 + 64, qt, :], asb[:], identb[:])
                        nc.vector.tensor_copy(outT[:, i, :], ptp[:])


            # ---- Phases D + E ----  (atpool closed; weights streamed)
            with (
                tc.tile_pool(name="phE", bufs=1) as epool,
                tc.tile_pool(name="psE", bufs=1, space="PSUM") as psE,
            ):
                aT = epool.tile([128, NFT + 2, QS], FP8, tag="aT", name="aT")
                nc.gpsimd.memset(aT[:, NFT, :], SG)
                nc.gpsimd.memset(aT[:, NFT + 1, :], 0.0)
                w2a = epool.tile([128, 17, 2, D], FP8, tag="w2a", name="w2a")
                w2b = epool.tile([128, 17, 2, D], FP8, tag="w2b", name="w2b")
                xres2 = epool.tile([128, NQT, D], F32, tag="xresd",
                                   name="xresd")
                nc.sync.dma_start(xres2[:], d_xres[:])
                w1cs = []
                for c in range(3):
                    w1c = epool.tile([128, 8, 2, 512], FP8, tag="w1c",
                                     name="w1c", bufs=3)
                    nc.sync.dma_start(w1c[:], d_w1[:, c, :, :, :])
                    w1cs.append(w1c)
                nc.sync.dma_start(w2a[:], d_w2[:, 0:17, :, :])

                with tc.tile_pool(name="phD", bufs=1) as dpool:
                    s1d = dpool.tile([128, NQT], F32, tag="s1d", name="s1d")
                    s2d = dpool.tile([128, NQT], F32, tag="s2d", name="s2d")
                    rstd2 = dpool.tile([128, NQT], F32, tag="rstd2",
                                       name="rstd2")
                    nmr2 = dpool.tile([128, NQT], F32, tag="nmr2", name="nmr2")
                    for qt in range(NQT):
                        pd = psE.tile([128, 1024], F32, tag="mme",
                                      name="pd", bufs=2)
                        for half in range(2):
                            for j in range(4):
                                nc.tensor.matmul(
                                    pd[:, half * 512:(half + 1) * 512],
                                    outT[:, 2 * j:2 * j + 2,
                                         qt * 128:(qt + 1) * 128],
                                    wo[:, j, :, half * 512:(half + 1) * 512],
                                    start=(j == 0), stop=(j == 3),
                                    perf_mode=PM.DoubleRow)
                        nc.vector.scalar_tensor_tensor(
                            x2[:, qt, :], pd[:], 1.0 / (SW * SA),
                            xres2[:, qt, :], op0=OP.mult, op1=OP.add)
                        ln_stats(dpool, x2[:, qt, :], s1d[:, qt:qt + 1],
                                 s2d[:, qt:qt + 1])
                    ln_finalize(dpool, s1d[:], s2d[:], NQT, rstd2[:], nmr2[:],
                                "lnfD")
                    for qt in range(NQT):
                        ln_norm_transpose(dpool, x2[:, qt, :],
                                          rstd2[:, qt:qt + 1],
                                          nmr2[:, qt:qt + 1], h2T, qt * 128,
                                          ddstT=dh2T, psp=psE)

                for c in range(8):
                    if c < 3:
                        w1c = w1cs[c]
                    else:
                        w1c = epool.tile([128, 8, 2, 512], FP8, tag="w1c",
                                         name="w1c", bufs=3)
                        nc.sync.dma_start(w1c[:], d_w1[:, c, :, :, :])
                        if c == 6:
                            nc.sync.dma_start(w2b[:], d_w2[:, 17:34, :, :])
                    for fp in range(2 * c, 2 * c + 2):
                        pa = psE.tile([128, 1024], F32, tag="mme",
                                      name="pa", bufs=2)
                        for half in range(2):
                            ft = 2 * fp + half
                            fl = (ft % 4) * 128
                            sl = slice(half * 512, (half + 1) * 512)
                            for j in range(12):
                                if j < 8:
                                    lhs = w1c[:, j, :, fl:fl + 128]
                                    rhs = h2T[:, 2 * (j % 4):2 * (j % 4) + 2, :]
                                else:
                                    lhs = w1c[:, j - 8, :, fl:fl + 128]
                                    rhs = dh2T[:, 2 * (j - 8):2 * (j - 8) + 2, :]
                                nc.tensor.matmul(
                                    pa[:, sl], lhs, rhs,
                                    start=(j == 0), stop=(j == 11),
                                    perf_mode=PM.DoubleRow)
                        for half in range(2):
                            ft = 2 * fp + half
                            gsc = epool.tile([128, 512], BF16, tag="gsc",
                                             name="gsc", bufs=3)
                            nc.scalar.activation(
                                gsc[:], pa[:, half * 512:(half + 1) * 512],
                                ACT.Gelu_apprx_tanh, bias=b1sb[:, ft:ft + 1],
                                scale=1.0 / SW)
                            nc.gpsimd.tensor_scalar(
                                aT[:, ft, :], gsc[:], SG, None, op0=OP.mult)
                for qp in range(2):
                    pys = [psE.tile([128, 1024], F32, tag="mme",
                                    name=f"pyw{q}", bufs=2)
                           for q in range(2)]
                    for j in range(34):
                        w2t = w2a if j < 17 else w2b
                        jl = j % 17
                        for q01 in range(2):
                            qt = 2 * qp + q01
                            for half in range(2):
                                nc.tensor.matmul(
                                    pys[q01][:, half * 512:(half + 1) * 512],
                                    aT[:, 2 * (j % 17):2 * (j % 17) + 2,
                                       qt * 128:(qt + 1) * 128],
                                    w2t[:, jl, :, half * 512:(half + 1) * 512],
                                    start=(j == 0), stop=(j == 33),
                                    perf_mode=PM.DoubleRow)
                    for q01 in range(2):
                        qt = 2 * qp + q01
                        ysb = epool.tile([128, D], F32, tag="ysb", name="ysb",
                                         bufs=2)
                        nc.vector.scalar_tensor_tensor(
                            ysb[:], pys[q01][:], 1.0 / (SW * SG), x2[:, qt, :],
                            op0=OP.mult, op1=OP.add)
                        nc.sync.dma_start(d_y[:, qt, :], ysb[:])

    nc.compile()
    return nc


def _gelu_tanh(x):
    x = x.astype(np.float64)
    return 0.5 * x * (1.0 + np.tanh(np.sqrt(2.0 / np.pi) * (x + 0.044715 * x ** 3)))


_F8 = ml_dtypes.float8_e4m3


def _dr_layout(w):
    """[Din, M] f32 -> [128, Din//256, 2, M] (no quantization)."""
    din, m = w.shape
    return w.reshape(din // 256, 2, 128, m).transpose(2, 0, 1, 3)


def _pack_dr(w):
    return np.ascontiguousarray(_dr_layout(w)).astype(_F8)


def _pack_dr_comp(w):
    """fp8 main + fp8 residual slots along the j axis."""
    q1 = w.astype(_F8)
    r = w - q1.astype(np.float32)
    main = _dr_layout(q1.astype(np.float32))
    resid = _dr_layout(r.astype(_F8).astype(np.float32))
    return np.ascontiguousarray(
        np.concatenate([main, resid], axis=1)).astype(_F8)


def _rowtile(a, n):
    m = a.shape[1]
    return np.ascontiguousarray(a.reshape(n, 128, m).transpose(1, 0, 2))


def kernel(x, torus_dist, time_emb, mask, ln1_g, ln1_b, Wqkv, Wout,
           torus_scale, ln2_g, ln2_b, W1, b1, W2, b2, Wt, bt):
    x = np.asarray(x, np.float32)
    torus_dist = np.asarray(torus_dist, np.float32)
    time_emb = np.asarray(time_emb, np.float32)
    mask = np.asarray(mask)
    Wqkv = np.asarray(Wqkv, np.float32)
    sc_arr = np.asarray(torus_scale, np.float32)

    allmask = bool(np.all(mask))
    key = f"nc_{allmask}"
    if key not in _CACHED:
        _CACHED[key] = _build(allmask=allmask)
    nc = _CACHED[key]

    bf = ml_dtypes.bfloat16

    tp = (_gelu_tanh(time_emb) @ np.asarray(Wt, np.float64)
          + np.asarray(bt, np.float64))          # [B, 2D]
    scale, shift = tp[:, :D], tp[:, D:]
    g_eff = (np.asarray(ln1_g, np.float64)[None, :] * (1.0 + scale))
    b_eff = (np.asarray(ln1_b, np.float64)[None, :] * (1.0 + scale) + shift)

    Wq_r = np.asarray(Wqkv[:, 0:D], np.float64) / np.sqrt(HD)
    Wk_r = np.asarray(Wqkv[:, D:2 * D], np.float64)
    Wv_r = np.asarray(Wqkv[:, 2 * D:3 * D], np.float64)
    W1_r = np.asarray(W1, np.float64)
    g2 = np.asarray(ln2_g, np.float64)
    b2ln = np.asarray(ln2_b, np.float64)

    wq_b, wk_b, wv_b, bq_b, bv_b = [], [], [], [], []
    for b_ in range(B):
        ge = g_eff[b_][:, None]
        be = b_eff[b_]
        wq_b.append(_pack_dr((ge * Wq_r * SW).astype(np.float32)))
        wk_b.append(_pack_dr((ge * Wk_r * SW).astype(np.float32)))
        wv_b.append(_pack_dr((ge * Wv_r * SW).astype(np.float32)))
        bq = (be @ Wq_r).astype(np.float32)
        bq_b.append(np.ascontiguousarray(bq.reshape(NDT, 128).T))
        bv = (be @ Wv_r).astype(np.float32)
        bv_b.append(np.ascontiguousarray(np.tile(bv[None, :], (128, 1))))
    wo_dr = _pack_dr((np.asarray(Wout, np.float64) * SW).astype(np.float32))
    w1_dr = _pack_dr_comp((g2[:, None] * W1_r * SW).astype(np.float32))
    # repack to ft-chunk-major for streamed loads: [128, chunk, j, t, 512]
    w1_dr = np.ascontiguousarray(
        w1_dr.reshape(128, 8, 2, 8, 512).transpose(0, 3, 1, 2, 4))
    w2_aug = np.zeros((F + 256, D), np.float32)
    w2_aug[:F] = (np.asarray(W2, np.float64) * SW).astype(np.float32)
    w2_aug[F] = (np.asarray(b2, np.float64) * SW * SG).astype(np.float32)
    w2_dr = _pack_dr_comp(w2_aug)
    b1sb_eff = (np.asarray(b1, np.float64) + b2ln @ W1_r).astype(np.float32)
    b1sb = np.ascontiguousarray(b1sb_eff.reshape(NFT, 128).T)

    idh = np.zeros((128, H, 2, 128), np.float32)
    sc_full = np.broadcast_to(sc_arr, (H,)).astype(np.float32)
    for h in range(H):
        np.fill_diagonal(idh[:, h, 0, :], sc_full[h])
    iddr = idh.astype(_F8)

    km_full = np.where(mask, 0.0, -88.0).astype(np.float32)   # [B, L]

    in_maps = []
    for c in range(8):
        b_, qs_ = c // NC_PER_B, c % NC_PER_B
        rows = slice(qs_ * QS, (qs_ + 1) * QS)
        xt = _rowtile(x[b_].astype(bf), NLT)
        xres = _rowtile(np.ascontiguousarray(x[b_, rows]), NQT
                        ).astype(np.float32)
        torT = torus_dist[0, rows, :].T.astype(np.float32)    # [L, QS]
        tor = np.zeros((128, NLT + 1, QS), _F8)
        tor[:, :NLT, :] = _rowtile(-torT, NLT).astype(_F8)
        kmt = np.ascontiguousarray(
            (km_full[b_] + ESH).reshape(NLT, 128).T)
        in_maps.append({
            "xt": xt, "xres": xres, "tor": tor, "iddr": iddr,
            "wq": wq_b[b_], "wk": wk_b[b_], "wv": wv_b[b_],
            "wo": wo_dr, "w1": w1_dr, "w2": w2_dr,
            "biasq": bq_b[b_], "b1sb": b1sb, "km": kmt,
            "bvrep": bv_b[b_],
        })

    import os
    trace = bool(int(os.environ.get("DENOISER_TRACE", "0")))
    res = run_bass_kernel_spmd(nc, in_maps, core_ids=list(range(8)), trace=trace)
    _CACHED["last_results"] = res

    out = np.empty((B, L, D), np.float32)
    for c in range(8):
        b_, qs_ = c // NC_PER_B, c % NC_PER_B
        y = res.results[c]["y"]
        out[b_, qs_ * QS:(qs_ + 1) * QS, :] = (
            y.transpose(1, 0, 2).reshape(QS, D))
    return out

